# revision 32
# baseline (speedup 1.0000x reference)
"""MoE FFN (grouped top-1 routing, SwiGLU experts) on 8 Trainium2 NeuronCores.

Strategy (expert-parallel, per sharding hint):
  - Host computes the (tiny) routers: sigmoid(x @ macro_w) -> top-1 group of 4;
    within the selected group both 2 experts are active with sigmoid-normalized
    weights. Router cost is ~25 MFLOP -> negligible.
  - Tokens are dispatched by routed group ("all-to-all" staged host-side into
    per-core input maps). Core c owns expert c (group c//2); it receives the
    tokens of its group, padded to capacity C, plus its expert's weights.
  - Per-expert weight w[t,e] is folded into the up-projection input on the host
    (x*w), so the device output is already weighted; host adds the two expert
    partials of each group and scatters back to token order.
  - Device kernel: Y^T = down^T @ (silu(gate^T X^T) * (up^T Xw^T)); features on
    SBUF partitions, tokens on the free dim, bf16 in / bf16 out, fp32 PSUM.

V2 performance notes (vs the 129.5us baseline):
  - All DRAM operands are pre-tiled on the host into the exact SBUF layout so
    every DMA is a few large contiguous descriptors (4-8KB/partition) instead
    of ~26k 0.25-1KB ones.
  - Weight stream is split into 512KB chunks issued in consumption order on the
    sync queue; x on gpsimd queue; first matmul can start ~1.5us after preamble.
  - A short burst of dummy matmuls warms the PE HAM clock-gate during the
    initial DMA fill so real matmuls run at 2.4GHz from the start.
  - cc-outer loop: the down-projection of token-chunk 0 runs while gate/up of
    chunk 1 still streams, spreading output DMAs and shrinking the tail.
"""

import math

import ml_dtypes
import numpy as np

import concourse.bass as bass  # noqa: F401  (bass types via bacc)
import concourse.mybir as mybir
import concourse.tile as tile
from concourse import bacc
from concourse.bass_utils import run_bass_kernel_spmd

P = 128
D_MODEL = 1024
FFN_DIM = 2048
NUM_EXPERTS = 8
NUM_GROUPS = 4
EPS = 1e-9

F32 = mybir.dt.float32
BF16 = mybir.dt.bfloat16

N_CORES = 8
C_CAP = 1024  # max token capacity per core per round (SBUF-bounded)

DO = D_MODEL // P  # 8 k-tiles over D
FO = FFN_DIM // P  # 16 f-tiles over F
# gate/up weight-stream chunk widths over F (f-columns); first chunks small so
# the first matmul chain can start early, big later chunks amortize the
# per-trigger ring overhead (~0.65us each).
FCHUNKS = (256, 256, 512, 512, 512)
DJ = 4             # down-weight chunks (2 d-tiles each)

N_WARM = 12        # dummy warm-up matmuls (HAM clock-gate)

_BUILD_CACHE: dict[tuple, object] = {}
LAST_RESULTS = None  # stashed BassKernelResults for test harnesses


def _build(C: int, cw: tuple):
    """Bass/Tile program for one expert: [D,C] tokens + expert weights -> [D,C].

    cw: per-chunk token widths (sum == C, each <= 512, multiples of 8)."""
    nch = len(cw)
    assert sum(cw) == C and all(w <= 512 and w % 8 == 0 for w in cw)
    chunk = max(cw)

    nc = bacc.Bacc(
        "TRN2",
        target_bir_lowering=False,
        debug=False,
        enable_asserts=False,
        num_devices=N_CORES,
    )
    # Pre-tiled DRAM layouts (host produces these exactly):
    #   xt/xwt: [P, DO, C]            (contiguous per partition: DO*C)
    #   gw/uw:  per-chunk [P, DO, w]  (per (chunk,p): DO*w contiguous)
    #   dw:     [DJ, P, 2, FO, P]     (per (j,p): 2*FO*P contiguous)
    #   yt:     [DO, P, C] bf16 out
    xt = nc.dram_tensor("xt", [P, DO, C], BF16, kind="ExternalInput").ap()
    wrow = nc.dram_tensor("wrow", [1, C], F32, kind="ExternalInput").ap()
    gws_d = [
        nc.dram_tensor(f"gw{i}", [P, DO, w], BF16, kind="ExternalInput").ap()
        for i, w in enumerate(FCHUNKS)
    ]
    uws_d = [
        nc.dram_tensor(f"uw{i}", [P, DO, w], BF16, kind="ExternalInput").ap()
        for i, w in enumerate(FCHUNKS)
    ]
    dw = nc.dram_tensor("dw", [DJ, P, 2, FO, P], BF16, kind="ExternalInput").ap()
    yt = nc.dram_tensor("yt", [DO, P, C], BF16, kind="ExternalOutput").ap()

    c0s = [sum(cw[:i]) for i in range(nch)]
    csl = [slice(c0s[cc], c0s[cc] + cw[cc]) for cc in range(nch)]
    with tile.TileContext(nc) as tc:
        with (
            tc.tile_pool(name="xp", bufs=1) as xp,
            tc.tile_pool(name="wp", bufs=1) as wp,
            tc.tile_pool(name="sp", bufs=4) as sp,
            tc.tile_pool(name="yp", bufs=4) as yp,
            tc.tile_pool(name="pw", bufs=1, space="PSUM") as pwp,
            tc.tile_pool(name="pg", bufs=3, space="PSUM") as pgp,
            tc.tile_pool(name="pu", bufs=2, space="PSUM") as pup,
            tc.tile_pool(name="pd", bufs=2, space="PSUM") as pdp,
        ):
            # ---- PE warm-up: dummy matmuls on a zeroed tile (no DMA deps) ----
            if N_WARM:
                warm = xp.tile([P, 512], BF16, tag="warm")
                nc.vector.memset(warm[:], 0.0)
                wps = pwp.tile([P, 256], F32, tag="wps")
                for i in range(N_WARM):
                    nc.tensor.matmul(
                        wps[:], warm[:, 0:128], warm[:, 0:256], start=True, stop=True
                    )

            # ---- input DMA streams ----
            # Per-core DMA budget is ~358GB/s; the early phase needs x + the
            # first gate/up chunks fastest. Split: x + up weights on the
            # gpsimd ring, gate + down weights (+ outputs later) on the sync
            # ring; the scalar ring keeps only silu work.
            xts = xp.tile([P, DO, C], BF16, tag="xt")
            gts = []
            uts = []
            for j in range(0, 6, 2):
                nc.gpsimd.dma_start(xts[:, j : j + 2], xt[:, j : j + 2])
            for i, w in enumerate(FCHUNKS):
                gt = wp.tile([P, DO, w], BF16, tag=f"gt{i}")
                nc.sync.dma_start(gt[:], gws_d[i])
                ut = wp.tile([P, DO, w], BF16, tag=f"ut{i}")
                uts.append(ut)
                gts.append(gt)
                if i == 0:
                    nc.gpsimd.dma_start(xts[:, 6:DO], xt[:, 6:DO])
                nc.gpsimd.dma_start(ut[:], uws_d[i])
            dts = []
            for j in range(DJ):
                dt_ = wp.tile([P, 2, FO, P], BF16, tag=f"dt{j}")
                nc.sync.dma_start(dt_[:], dw[j])
                dts.append(dt_)
            # per-token output scale row: DMA to partition 0, broadcast to all
            # (first needed by the down-projection, ~45us in)
            w2r = xp.tile([1, C], F32, tag="w2r")
            nc.gpsimd.dma_start(w2r[:], wrow[:])
            w2s = xp.tile([P, C], F32, tag="w2s")
            nc.gpsimd.partition_broadcast(w2s[:], w2r[:])

            hs = xp.tile([P, FO, C], BF16, tag="h")

            # fo -> (gate/up chunk index, f-slice within the chunk)
            fo_map = []
            for i, w in enumerate(FCHUNKS):
                for fl in range(w // P):
                    fo_map.append((i, slice(fl * P, (fl + 1) * P)))
            assert len(fo_map) == FO

            for cc in range(nch):
                cs = csl[cc]
                w_c = cw[cc]
                for fo in range(FO):
                    i, fsl = fo_map[fo]
                    psg = pgp.tile([P, chunk], F32, tag="psg", name=f"psg_{fo}_{cc}")
                    psu = pup.tile([P, chunk], F32, tag="psu", name=f"psu_{fo}_{cc}")
                    for do in range(DO):
                        nc.tensor.matmul(
                            psg[:, :w_c],
                            gts[i][:, do, fsl],
                            xts[:, do, cs],
                            start=(do == 0),
                            stop=(do == DO - 1),
                        )
                    for do in range(DO):
                        nc.tensor.matmul(
                            psu[:, :w_c],
                            uts[i][:, do, fsl],
                            xts[:, do, cs],
                            start=(do == 0),
                            stop=(do == DO - 1),
                        )
                    sg = sp.tile([P, chunk], F32, tag="sg")
                    nc.scalar.activation(
                        sg[:, :w_c], psg[:, :w_c], mybir.ActivationFunctionType.Silu
                    )
                    nc.vector.tensor_mul(
                        out=hs[:, fo, cs], in0=sg[:, :w_c], in1=psu[:, :w_c]
                    )
                # down-projection for this token chunk (h complete for cs)
                for j in range(DJ):
                    for k in range(2):
                        do = 2 * j + k
                        psy = pdp.tile([P, chunk], F32, tag="psy", name=f"psy_{do}_{cc}")
                        for fo in range(FO):
                            nc.tensor.matmul(
                                psy[:, :w_c],
                                dts[j][:, k, fo],
                                hs[:, fo, cs],
                                start=(fo == 0),
                                stop=(fo == FO - 1),
                            )
                        yo = yp.tile([P, chunk], BF16, tag="yo")
                        nc.vector.tensor_mul(
                            out=yo[:, :w_c], in0=psy[:, :w_c], in1=w2s[:, cs]
                        )
                        nc.sync.dma_start(yt[do, :, cs], yo[:, :w_c])
    nc.finalize()
    return nc


def _get_program(C: int, cw: tuple):
    key = (C, cw)
    if key not in _BUILD_CACHE:
        _BUILD_CACHE[key] = _build(C, cw)
    return _BUILD_CACHE[key]


def _sigmoid(z):
    return 1.0 / (1.0 + np.exp(-z))


def _route(xf32, macro_w, micro_w):
    """Host routers in float64. Returns group index per token and per-token
    weights for the 2 experts of the selected group (float32)."""
    xf = xf32.astype(np.float64)
    ms = _sigmoid(xf @ macro_w.astype(np.float64))  # [T, G]
    g_sel = np.argmax(ms, axis=1)
    T = xf.shape[0]
    mval = ms[np.arange(T), g_sel]
    mv = mval / (mval + EPS)

    w2 = np.zeros((T, 2), np.float64)
    for g in range(NUM_GROUPS):
        idx = np.nonzero(g_sel == g)[0]
        if idx.size == 0:
            continue
        s = _sigmoid(xf[idx] @ micro_w[g].astype(np.float64))  # [n, 2]
        denom = np.maximum(s[:, 0], s[:, 1]) + np.minimum(s[:, 0], s[:, 1]) + EPS
        w2[idx, 0] = mv[idx] * s[:, 0] / denom
        w2[idx, 1] = mv[idx] * s[:, 1] / denom
    return g_sel, w2.astype(np.float32)


def _pick_capacity(n: int):
    """Token capacity C and per-chunk widths. Chunks must be <=512 and
    multiples of 8 (16B SBUF lines); the first chunk is 16-aligned."""
    n = max(n, 64)
    nch = (n + 511) // 512
    c1 = -(-(-(-n // nch)) // 16) * 16
    cws = []
    rem = n
    for i in range(nch):
        w = min(c1, -(-rem // 8) * 8) if i == nch - 1 else c1
        cws.append(w)
        rem -= w
    cws = [w for w in cws if w > 0]
    return sum(cws), tuple(cws)


def _tile_gate_up(w):
    """[D, F] f32 -> dict of per-chunk [P, DO, w_i] bf16 in SBUF order."""
    t = w.reshape(DO, P, FFN_DIM).transpose(1, 0, 2)  # [P, DO, F]
    out = {}
    f0 = 0
    for i, wd in enumerate(FCHUNKS):
        out[i] = np.ascontiguousarray(t[:, :, f0 : f0 + wd]).astype(
            ml_dtypes.bfloat16
        )
        f0 += wd
    return out


def _tile_down(w):
    """[F, D] f32 -> [DJ, P, 2, FO, P] bf16 in SBUF order."""
    t = w.reshape(FO, P, DJ, 2, P).transpose(2, 1, 3, 0, 4)
    return np.ascontiguousarray(t).astype(ml_dtypes.bfloat16)


def _tile_x(xg, C):
    """[n, D] f32 -> [P, DO, C] bf16 (zero-padded to C tokens)."""
    out = np.zeros((P, DO, C), ml_dtypes.bfloat16)
    n = xg.shape[0]
    if n:
        # token c, feature d = do*P + p  ->  out[p, do, c]
        t = xg.T.reshape(DO, P, n).transpose(1, 0, 2)
        out[:, :, :n] = t.astype(ml_dtypes.bfloat16)
    return out


def kernel(x, macro_w, micro_w, gate_w, up_w, down_w):
    global LAST_RESULTS
    x = np.asarray(x)
    B, S, D = x.shape
    T = B * S
    xf = np.ascontiguousarray(x.reshape(T, D).astype(np.float32, copy=False))

    g_sel, w2 = _route(xf, np.asarray(macro_w), np.asarray(micro_w))
    idx_by_g = [np.nonzero(g_sel == g)[0] for g in range(NUM_GROUPS)]
    max_n = max(ix.size for ix in idx_by_g)

    n_rounds = max(1, math.ceil(max_n / C_CAP))
    if n_rounds > 1:
        C, cw = C_CAP, (512, 512)
    else:
        C, cw = _pick_capacity(max_n)
    nc = _get_program(C, cw)

    gate_w = np.asarray(gate_w, np.float32)
    up_w = np.asarray(up_w, np.float32)
    down_w = np.asarray(down_w, np.float32)
    gw_t = [_tile_gate_up(gate_w[e]) for e in range(NUM_EXPERTS)]
    uw_t = [_tile_gate_up(up_w[e]) for e in range(NUM_EXPERTS)]
    dw_t = [_tile_down(down_w[e]) for e in range(NUM_EXPERTS)]

    y = np.zeros((T, D), np.float32)
    for r in range(n_rounds):
        in_maps = []
        round_idx = []
        for c in range(N_CORES):
            g = c // 2
            j = c % 2  # local expert within group
            ix = idx_by_g[g][r * C_CAP : r * C_CAP + C]
            round_idx.append(ix)
            xg = xf[ix]
            wr = np.zeros((1, C), np.float32)
            wr[0, : ix.size] = w2[ix, j]
            im = {
                "xt": _tile_x(xg, C),
                "wrow": wr,
                "dw": dw_t[c],
            }
            for i in range(len(FCHUNKS)):
                im[f"gw{i}"] = gw_t[c][i]
                im[f"uw{i}"] = uw_t[c][i]
            in_maps.append(im)
        res = run_bass_kernel_spmd(nc, in_maps, core_ids=list(range(N_CORES)))
        LAST_RESULTS = res
        for g in range(NUM_GROUPS):
            ix = round_idx[2 * g]
            if ix.size:
                # yt: [DO, P, C] bf16 -> [D, C] f32
                y0 = res.results[2 * g]["yt"].astype(np.float32)
                y1 = res.results[2 * g + 1]["yt"].astype(np.float32)
                ysum = (y0 + y1).reshape(D, C)
                y[ix] = ysum[:, : ix.size].T
    return y.reshape(B, S, D)


# revision 34
# speedup vs baseline: 1.0745x; 1.0745x over previous
"""MoE FFN (grouped top-1 routing, SwiGLU experts) on 8 Trainium2 NeuronCores.

Strategy (expert-parallel, per sharding hint):
  - Host computes the (tiny) routers: sigmoid(x @ macro_w) -> top-1 group of 4;
    within the selected group both 2 experts are active with sigmoid-normalized
    weights. Router cost is ~25 MFLOP -> negligible.
  - Tokens are dispatched by routed group ("all-to-all" staged host-side into
    per-core input maps). Core c owns expert c (group c//2); it receives the
    tokens of its group, padded to capacity C, plus its expert's weights.
  - Per-expert weight w[t,e] is folded into the up-projection input on the host
    (x*w), so the device output is already weighted; host adds the two expert
    partials of each group and scatters back to token order.
  - Device kernel: Y^T = down^T @ (silu(gate^T X^T) * (up^T Xw^T)); features on
    SBUF partitions, tokens on the free dim, bf16 in / bf16 out, fp32 PSUM.

V2 performance notes (vs the 129.5us baseline):
  - All DRAM operands are pre-tiled on the host into the exact SBUF layout so
    every DMA is a few large contiguous descriptors (4-8KB/partition) instead
    of ~26k 0.25-1KB ones.
  - Weight stream is split into 512KB chunks issued in consumption order on the
    sync queue; x on gpsimd queue; first matmul can start ~1.5us after preamble.
  - A short burst of dummy matmuls warms the PE HAM clock-gate during the
    initial DMA fill so real matmuls run at 2.4GHz from the start.
  - cc-outer loop: the down-projection of token-chunk 0 runs while gate/up of
    chunk 1 still streams, spreading output DMAs and shrinking the tail.
"""

import math

import ml_dtypes
import numpy as np

import concourse.bass as bass  # noqa: F401  (bass types via bacc)
import concourse.mybir as mybir
import concourse.tile as tile
from concourse import bacc
from concourse.bass_utils import run_bass_kernel_spmd

P = 128
D_MODEL = 1024
FFN_DIM = 2048
NUM_EXPERTS = 8
NUM_GROUPS = 4
EPS = 1e-9

F32 = mybir.dt.float32
BF16 = mybir.dt.bfloat16

N_CORES = 8
C_CAP = 1024  # max token capacity per core per round (SBUF-bounded)

DO = D_MODEL // P  # 8 k-tiles over D
FO = FFN_DIM // P  # 16 f-tiles over F
# gate/up weight-stream chunk widths over F (f-columns); first chunks small so
# the first matmul chain can start early, big later chunks amortize the
# per-trigger ring overhead (~0.65us each).
FCHUNKS = (256, 256, 512, 512, 512)
DJ = 4             # down-weight chunks (2 d-tiles each)

N_WARM = 0         # dummy warm-up matmuls (HAM clock-gate)

_BUILD_CACHE: dict[tuple, object] = {}
LAST_RESULTS = None  # stashed BassKernelResults for test harnesses


def _build(C: int, cw: tuple):
    """Bass/Tile program for one expert: [D,C] tokens + expert weights -> [D,C].

    cw: per-chunk token widths (sum == C, each <= 512, multiples of 8)."""
    nch = len(cw)
    assert sum(cw) == C and all(w <= 512 and w % 8 == 0 for w in cw)
    chunk = max(cw)

    nc = bacc.Bacc(
        "TRN2",
        target_bir_lowering=False,
        debug=False,
        enable_asserts=False,
        num_devices=N_CORES,
    )
    # Pre-tiled DRAM layouts (host produces these exactly):
    #   xt/xwt: [P, DO, C]            (contiguous per partition: DO*C)
    #   gw/uw:  per-chunk [P, DO, w]  (per (chunk,p): DO*w contiguous)
    #   dw:     [DJ, P, 2, FO, P]     (per (j,p): 2*FO*P contiguous)
    #   yt:     [DO, P, C] bf16 out
    xt = nc.dram_tensor("xt", [P, DO, C], BF16, kind="ExternalInput").ap()
    wrow = nc.dram_tensor("wrow", [1, C], F32, kind="ExternalInput").ap()
    gws_d = [
        nc.dram_tensor(f"gw{i}", [P, DO, w], BF16, kind="ExternalInput").ap()
        for i, w in enumerate(FCHUNKS)
    ]
    uws_d = [
        nc.dram_tensor(f"uw{i}", [P, DO, w], BF16, kind="ExternalInput").ap()
        for i, w in enumerate(FCHUNKS)
    ]
    dw = nc.dram_tensor("dw", [DJ, P, 2, FO, P], BF16, kind="ExternalInput").ap()
    yt = nc.dram_tensor("yt", [DO, P, C], BF16, kind="ExternalOutput").ap()

    c0s = [sum(cw[:i]) for i in range(nch)]
    csl = [slice(c0s[cc], c0s[cc] + cw[cc]) for cc in range(nch)]
    with tile.TileContext(nc) as tc:
        with (
            tc.tile_pool(name="xp", bufs=1) as xp,
            tc.tile_pool(name="wp", bufs=1) as wp,
            tc.tile_pool(name="sp", bufs=4) as sp,
            tc.tile_pool(name="yp", bufs=4) as yp,
            tc.tile_pool(name="pw", bufs=1, space="PSUM") as pwp,
            tc.tile_pool(name="pg", bufs=3, space="PSUM") as pgp,
            tc.tile_pool(name="pu", bufs=2, space="PSUM") as pup,
            tc.tile_pool(name="pd", bufs=2, space="PSUM") as pdp,
        ):
            # ---- PE warm-up: dummy matmuls on a zeroed tile (no DMA deps) ----
            if N_WARM:
                warm = xp.tile([P, 512], BF16, tag="warm")
                nc.vector.memset(warm[:], 0.0)
                wps = pwp.tile([P, 256], F32, tag="wps")
                for i in range(N_WARM):
                    nc.tensor.matmul(
                        wps[:], warm[:, 0:128], warm[:, 0:256], start=True, stop=True
                    )

            # ---- input DMA streams ----
            # The sync ring is the fast DMA path: all weights (gate/up
            # interleaved in consumption order, then down) and later the
            # outputs go there. x rides the gpsimd ring in parallel.
            xts = xp.tile([P, DO, C], BF16, tag="xt")
            half = DO // 2
            nc.gpsimd.dma_start(xts[:, 0:half], xt[:, 0:half])
            nc.gpsimd.dma_start(xts[:, half:DO], xt[:, half:DO])
            # per-token output scale row: DMA to partition 0, broadcast to all
            # (first needed by the down-projection, ~45us in)
            w2r = xp.tile([1, C], F32, tag="w2r")
            nc.gpsimd.dma_start(w2r[:], wrow[:])
            w2s = xp.tile([P, C], F32, tag="w2s")
            nc.gpsimd.partition_broadcast(w2s[:], w2r[:])
            gts = []
            uts = []
            for i, w in enumerate(FCHUNKS):
                gt = wp.tile([P, DO, w], BF16, tag=f"gt{i}")
                nc.sync.dma_start(gt[:], gws_d[i])
                ut = wp.tile([P, DO, w], BF16, tag=f"ut{i}")
                nc.sync.dma_start(ut[:], uws_d[i])
                gts.append(gt)
                uts.append(ut)
            dts = []
            for j in range(DJ):
                dt_ = wp.tile([P, 2, FO, P], BF16, tag=f"dt{j}")
                nc.sync.dma_start(dt_[:], dw[j])
                dts.append(dt_)

            hs = xp.tile([P, FO, C], BF16, tag="h")

            # fo -> (gate/up chunk index, f-slice within the chunk)
            fo_map = []
            for i, w in enumerate(FCHUNKS):
                for fl in range(w // P):
                    fo_map.append((i, slice(fl * P, (fl + 1) * P)))
            assert len(fo_map) == FO

            for cc in range(nch):
                cs = csl[cc]
                w_c = cw[cc]
                for fo in range(FO):
                    i, fsl = fo_map[fo]
                    psg = pgp.tile([P, chunk], F32, tag="psg", name=f"psg_{fo}_{cc}")
                    psu = pup.tile([P, chunk], F32, tag="psu", name=f"psu_{fo}_{cc}")
                    for do in range(DO):
                        nc.tensor.matmul(
                            psg[:, :w_c],
                            gts[i][:, do, fsl],
                            xts[:, do, cs],
                            start=(do == 0),
                            stop=(do == DO - 1),
                        )
                    for do in range(DO):
                        nc.tensor.matmul(
                            psu[:, :w_c],
                            uts[i][:, do, fsl],
                            xts[:, do, cs],
                            start=(do == 0),
                            stop=(do == DO - 1),
                        )
                    sg = sp.tile([P, chunk], F32, tag="sg")
                    nc.scalar.activation(
                        sg[:, :w_c], psg[:, :w_c], mybir.ActivationFunctionType.Silu
                    )
                    nc.vector.tensor_mul(
                        out=hs[:, fo, cs], in0=sg[:, :w_c], in1=psu[:, :w_c]
                    )
                # down-projection for this token chunk (h complete for cs)
                for j in range(DJ):
                    for k in range(2):
                        do = 2 * j + k
                        psy = pdp.tile([P, chunk], F32, tag="psy", name=f"psy_{do}_{cc}")
                        for fo in range(FO):
                            nc.tensor.matmul(
                                psy[:, :w_c],
                                dts[j][:, k, fo],
                                hs[:, fo, cs],
                                start=(fo == 0),
                                stop=(fo == FO - 1),
                            )
                        yo = yp.tile([P, chunk], BF16, tag="yo")
                        nc.vector.tensor_mul(
                            out=yo[:, :w_c], in0=psy[:, :w_c], in1=w2s[:, cs]
                        )
                        nc.sync.dma_start(yt[do, :, cs], yo[:, :w_c])
    nc.finalize()
    return nc


def _get_program(C: int, cw: tuple):
    key = (C, cw)
    if key not in _BUILD_CACHE:
        _BUILD_CACHE[key] = _build(C, cw)
    return _BUILD_CACHE[key]


def _sigmoid(z):
    return 1.0 / (1.0 + np.exp(-z))


def _route(xf32, macro_w, micro_w):
    """Host routers in float64. Returns group index per token and per-token
    weights for the 2 experts of the selected group (float32)."""
    xf = xf32.astype(np.float64)
    ms = _sigmoid(xf @ macro_w.astype(np.float64))  # [T, G]
    g_sel = np.argmax(ms, axis=1)
    T = xf.shape[0]
    mval = ms[np.arange(T), g_sel]
    mv = mval / (mval + EPS)

    w2 = np.zeros((T, 2), np.float64)
    for g in range(NUM_GROUPS):
        idx = np.nonzero(g_sel == g)[0]
        if idx.size == 0:
            continue
        s = _sigmoid(xf[idx] @ micro_w[g].astype(np.float64))  # [n, 2]
        denom = np.maximum(s[:, 0], s[:, 1]) + np.minimum(s[:, 0], s[:, 1]) + EPS
        w2[idx, 0] = mv[idx] * s[:, 0] / denom
        w2[idx, 1] = mv[idx] * s[:, 1] / denom
    return g_sel, w2.astype(np.float32)


def _pick_capacity(n: int):
    """Token capacity C and per-chunk widths. Chunks must be <=512 and
    multiples of 8 (16B SBUF lines); the first chunk is 16-aligned."""
    n = max(n, 64)
    nch = (n + 511) // 512
    c1 = -(-(-(-n // nch)) // 16) * 16
    cws = []
    rem = n
    for i in range(nch):
        w = min(c1, -(-rem // 8) * 8) if i == nch - 1 else c1
        cws.append(w)
        rem -= w
    cws = [w for w in cws if w > 0]
    return sum(cws), tuple(cws)


def _tile_gate_up(w):
    """[D, F] f32 -> dict of per-chunk [P, DO, w_i] bf16 in SBUF order."""
    t = w.reshape(DO, P, FFN_DIM).transpose(1, 0, 2)  # [P, DO, F]
    out = {}
    f0 = 0
    for i, wd in enumerate(FCHUNKS):
        out[i] = np.ascontiguousarray(t[:, :, f0 : f0 + wd]).astype(
            ml_dtypes.bfloat16
        )
        f0 += wd
    return out


def _tile_down(w):
    """[F, D] f32 -> [DJ, P, 2, FO, P] bf16 in SBUF order."""
    t = w.reshape(FO, P, DJ, 2, P).transpose(2, 1, 3, 0, 4)
    return np.ascontiguousarray(t).astype(ml_dtypes.bfloat16)


def _tile_x(xg, C):
    """[n, D] f32 -> [P, DO, C] bf16 (zero-padded to C tokens)."""
    out = np.zeros((P, DO, C), ml_dtypes.bfloat16)
    n = xg.shape[0]
    if n:
        # token c, feature d = do*P + p  ->  out[p, do, c]
        t = xg.T.reshape(DO, P, n).transpose(1, 0, 2)
        out[:, :, :n] = t.astype(ml_dtypes.bfloat16)
    return out


def kernel(x, macro_w, micro_w, gate_w, up_w, down_w):
    global LAST_RESULTS
    x = np.asarray(x)
    B, S, D = x.shape
    T = B * S
    xf = np.ascontiguousarray(x.reshape(T, D).astype(np.float32, copy=False))

    g_sel, w2 = _route(xf, np.asarray(macro_w), np.asarray(micro_w))
    idx_by_g = [np.nonzero(g_sel == g)[0] for g in range(NUM_GROUPS)]
    max_n = max(ix.size for ix in idx_by_g)

    n_rounds = max(1, math.ceil(max_n / C_CAP))
    if n_rounds > 1:
        C, cw = C_CAP, (512, 512)
    else:
        C, cw = _pick_capacity(max_n)
    nc = _get_program(C, cw)

    gate_w = np.asarray(gate_w, np.float32)
    up_w = np.asarray(up_w, np.float32)
    down_w = np.asarray(down_w, np.float32)
    gw_t = [_tile_gate_up(gate_w[e]) for e in range(NUM_EXPERTS)]
    uw_t = [_tile_gate_up(up_w[e]) for e in range(NUM_EXPERTS)]
    dw_t = [_tile_down(down_w[e]) for e in range(NUM_EXPERTS)]

    y = np.zeros((T, D), np.float32)
    for r in range(n_rounds):
        in_maps = []
        round_idx = []
        for c in range(N_CORES):
            g = c // 2
            j = c % 2  # local expert within group
            ix = idx_by_g[g][r * C_CAP : r * C_CAP + C]
            round_idx.append(ix)
            xg = xf[ix]
            wr = np.zeros((1, C), np.float32)
            wr[0, : ix.size] = w2[ix, j]
            im = {
                "xt": _tile_x(xg, C),
                "wrow": wr,
                "dw": dw_t[c],
            }
            for i in range(len(FCHUNKS)):
                im[f"gw{i}"] = gw_t[c][i]
                im[f"uw{i}"] = uw_t[c][i]
            in_maps.append(im)
        res = run_bass_kernel_spmd(nc, in_maps, core_ids=list(range(N_CORES)))
        LAST_RESULTS = res
        for g in range(NUM_GROUPS):
            ix = round_idx[2 * g]
            if ix.size:
                # yt: [DO, P, C] bf16 -> [D, C] f32
                y0 = res.results[2 * g]["yt"].astype(np.float32)
                y1 = res.results[2 * g + 1]["yt"].astype(np.float32)
                ysum = (y0 + y1).reshape(D, C)
                y[ix] = ysum[:, : ix.size].T
    return y.reshape(B, S, D)


# revision 36
# speedup vs baseline: 1.1407x; 1.0616x over previous
"""MoE FFN (grouped top-1 routing, SwiGLU experts) on 8 Trainium2 NeuronCores.

Strategy (expert-parallel, per sharding hint):
  - Host computes the (tiny) routers: sigmoid(x @ macro_w) -> top-1 group of 4;
    within the selected group both 2 experts are active with sigmoid-normalized
    weights. Router cost is ~25 MFLOP -> negligible.
  - Tokens are dispatched by routed group ("all-to-all" staged host-side into
    per-core input maps). Core c owns expert c (group c//2); it receives the
    tokens of its group, padded to capacity C, plus its expert's weights.
  - Per-expert weight w[t,e] is folded into the up-projection input on the host
    (x*w), so the device output is already weighted; host adds the two expert
    partials of each group and scatters back to token order.
  - Device kernel: Y^T = down^T @ (silu(gate^T X^T) * (up^T Xw^T)); features on
    SBUF partitions, tokens on the free dim, bf16 in / bf16 out, fp32 PSUM.

V2 performance notes (vs the 129.5us baseline):
  - All DRAM operands are pre-tiled on the host into the exact SBUF layout so
    every DMA is a few large contiguous descriptors (4-8KB/partition) instead
    of ~26k 0.25-1KB ones.
  - Weight stream is split into 512KB chunks issued in consumption order on the
    sync queue; x on gpsimd queue; first matmul can start ~1.5us after preamble.
  - A short burst of dummy matmuls warms the PE HAM clock-gate during the
    initial DMA fill so real matmuls run at 2.4GHz from the start.
  - cc-outer loop: the down-projection of token-chunk 0 runs while gate/up of
    chunk 1 still streams, spreading output DMAs and shrinking the tail.
"""

import math

import ml_dtypes
import numpy as np

import concourse.bass as bass  # noqa: F401  (bass types via bacc)
import concourse.mybir as mybir
import concourse.tile as tile
from concourse import bacc
from concourse.bass_utils import run_bass_kernel_spmd

P = 128
D_MODEL = 1024
FFN_DIM = 2048
NUM_EXPERTS = 8
NUM_GROUPS = 4
EPS = 1e-9

F32 = mybir.dt.float32
BF16 = mybir.dt.bfloat16

N_CORES = 8
C_CAP = 1024  # max token capacity per core per round (SBUF-bounded)

DO = D_MODEL // P  # 8 k-tiles over D
FO = FFN_DIM // P  # 16 f-tiles over F
# gate/up weight-stream chunk widths over F (f-columns); first chunks small so
# the first matmul chain can start early, big later chunks amortize the
# per-trigger ring overhead (~0.65us each).
FCHUNKS = (256, 256, 512, 512, 512)
DJ = 4             # down-weight chunks (2 d-tiles each)

N_WARM = 12        # dummy warm-up matmuls (HAM clock-gate)

_BUILD_CACHE: dict[tuple, object] = {}
LAST_RESULTS = None  # stashed BassKernelResults for test harnesses


def _build(C: int, cw: tuple):
    """Bass/Tile program for one expert: [D,C] tokens + expert weights -> [D,C].

    cw: per-chunk token widths (sum == C, each <= 512, multiples of 8)."""
    nch = len(cw)
    assert sum(cw) == C and all(w <= 512 and w % 8 == 0 for w in cw)
    chunk = max(cw)

    nc = bacc.Bacc(
        "TRN2",
        target_bir_lowering=False,
        debug=False,
        enable_asserts=False,
        num_devices=N_CORES,
    )
    # Pre-tiled DRAM layouts (host produces these exactly):
    #   xt/xwt: [P, DO, C]            (contiguous per partition: DO*C)
    #   gw/uw:  per-chunk [P, DO, w]  (per (chunk,p): DO*w contiguous)
    #   dw:     [DJ, P, 2, FO, P]     (per (j,p): 2*FO*P contiguous)
    #   yt:     [DO, P, C] bf16 out
    xt = nc.dram_tensor("xt", [P, DO, C], BF16, kind="ExternalInput").ap()
    wrow = nc.dram_tensor("wrow", [1, C], F32, kind="ExternalInput").ap()
    gws_d = [
        nc.dram_tensor(f"gw{i}", [P, DO, w], BF16, kind="ExternalInput").ap()
        for i, w in enumerate(FCHUNKS)
    ]
    uws_d = [
        nc.dram_tensor(f"uw{i}", [P, DO, w], BF16, kind="ExternalInput").ap()
        for i, w in enumerate(FCHUNKS)
    ]
    dw = nc.dram_tensor("dw", [DJ, P, 2, FO, P], BF16, kind="ExternalInput").ap()
    yt = nc.dram_tensor("yt", [DO, P, C], BF16, kind="ExternalOutput").ap()

    c0s = [sum(cw[:i]) for i in range(nch)]
    csl = [slice(c0s[cc], c0s[cc] + cw[cc]) for cc in range(nch)]
    with tile.TileContext(nc) as tc:
        with (
            tc.tile_pool(name="xp", bufs=1) as xp,
            tc.tile_pool(name="wp", bufs=1) as wp,
            tc.tile_pool(name="sp", bufs=4) as sp,
            tc.tile_pool(name="yp", bufs=4) as yp,
            tc.tile_pool(name="pw", bufs=1, space="PSUM") as pwp,
            tc.tile_pool(name="pg", bufs=3, space="PSUM") as pgp,
            tc.tile_pool(name="pu", bufs=2, space="PSUM") as pup,
            tc.tile_pool(name="pd", bufs=2, space="PSUM") as pdp,
        ):
            # ---- PE warm-up: dummy matmuls on a zeroed tile (no DMA deps) ----
            if N_WARM:
                warm = xp.tile([P, 512], BF16, tag="warm")
                nc.vector.memset(warm[:], 0.0)
                wps = pwp.tile([P, 256], F32, tag="wps")
                for i in range(N_WARM):
                    nc.tensor.matmul(
                        wps[:], warm[:, 0:128], warm[:, 0:256], start=True, stop=True
                    )

            # ---- input DMA streams ----
            # The sync ring is the only fast DMA path, so everything rides it
            # in consumption order: x first (interleaved with the first
            # gate/up chunks), then the weight stream, then (later) outputs.
            xts = xp.tile([P, DO, C], BF16, tag="xt")
            half = DO // 2
            nc.sync.dma_start(xts[:, 0:half], xt[:, 0:half])
            gts = []
            uts = []
            for i, w in enumerate(FCHUNKS):
                gt = wp.tile([P, DO, w], BF16, tag=f"gt{i}")
                nc.sync.dma_start(gt[:], gws_d[i])
                if i == 0:
                    nc.sync.dma_start(xts[:, half:DO], xt[:, half:DO])
                ut = wp.tile([P, DO, w], BF16, tag=f"ut{i}")
                nc.sync.dma_start(ut[:], uws_d[i])
                gts.append(gt)
                uts.append(ut)
            dts = []
            for j in range(DJ):
                dt_ = wp.tile([P, 2, FO, P], BF16, tag=f"dt{j}")
                nc.sync.dma_start(dt_[:], dw[j])
                dts.append(dt_)
            # per-token output scale row: DMA to partition 0 via the gpsimd
            # ring, broadcast to all partitions (first needed ~45us in)
            w2r = xp.tile([1, C], F32, tag="w2r")
            nc.gpsimd.dma_start(w2r[:], wrow[:])
            w2s = xp.tile([P, C], F32, tag="w2s")
            nc.gpsimd.partition_broadcast(w2s[:], w2r[:])

            hs = xp.tile([P, FO, C], BF16, tag="h")

            # fo -> (gate/up chunk index, f-slice within the chunk)
            fo_map = []
            for i, w in enumerate(FCHUNKS):
                for fl in range(w // P):
                    fo_map.append((i, slice(fl * P, (fl + 1) * P)))
            assert len(fo_map) == FO

            for cc in range(nch):
                cs = csl[cc]
                w_c = cw[cc]
                for fo in range(FO):
                    i, fsl = fo_map[fo]
                    psg = pgp.tile([P, chunk], F32, tag="psg", name=f"psg_{fo}_{cc}")
                    psu = pup.tile([P, chunk], F32, tag="psu", name=f"psu_{fo}_{cc}")
                    for do in range(DO):
                        nc.tensor.matmul(
                            psg[:, :w_c],
                            gts[i][:, do, fsl],
                            xts[:, do, cs],
                            start=(do == 0),
                            stop=(do == DO - 1),
                        )
                    for do in range(DO):
                        nc.tensor.matmul(
                            psu[:, :w_c],
                            uts[i][:, do, fsl],
                            xts[:, do, cs],
                            start=(do == 0),
                            stop=(do == DO - 1),
                        )
                    sg = sp.tile([P, chunk], F32, tag="sg")
                    nc.scalar.activation(
                        sg[:, :w_c], psg[:, :w_c], mybir.ActivationFunctionType.Silu
                    )
                    nc.vector.tensor_mul(
                        out=hs[:, fo, cs], in0=sg[:, :w_c], in1=psu[:, :w_c]
                    )
                # down-projection for this token chunk (h complete for cs)
                for j in range(DJ):
                    for k in range(2):
                        do = 2 * j + k
                        psy = pdp.tile([P, chunk], F32, tag="psy", name=f"psy_{do}_{cc}")
                        for fo in range(FO):
                            nc.tensor.matmul(
                                psy[:, :w_c],
                                dts[j][:, k, fo],
                                hs[:, fo, cs],
                                start=(fo == 0),
                                stop=(fo == FO - 1),
                            )
                        yo = yp.tile([P, chunk], BF16, tag="yo")
                        nc.vector.tensor_mul(
                            out=yo[:, :w_c], in0=psy[:, :w_c], in1=w2s[:, cs]
                        )
                        nc.sync.dma_start(yt[do, :, cs], yo[:, :w_c])
    nc.finalize()
    return nc


def _get_program(C: int, cw: tuple):
    key = (C, cw)
    if key not in _BUILD_CACHE:
        _BUILD_CACHE[key] = _build(C, cw)
    return _BUILD_CACHE[key]


def _sigmoid(z):
    return 1.0 / (1.0 + np.exp(-z))


def _route(xf32, macro_w, micro_w):
    """Host routers in float64. Returns group index per token and per-token
    weights for the 2 experts of the selected group (float32)."""
    xf = xf32.astype(np.float64)
    ms = _sigmoid(xf @ macro_w.astype(np.float64))  # [T, G]
    g_sel = np.argmax(ms, axis=1)
    T = xf.shape[0]
    mval = ms[np.arange(T), g_sel]
    mv = mval / (mval + EPS)

    w2 = np.zeros((T, 2), np.float64)
    for g in range(NUM_GROUPS):
        idx = np.nonzero(g_sel == g)[0]
        if idx.size == 0:
            continue
        s = _sigmoid(xf[idx] @ micro_w[g].astype(np.float64))  # [n, 2]
        denom = np.maximum(s[:, 0], s[:, 1]) + np.minimum(s[:, 0], s[:, 1]) + EPS
        w2[idx, 0] = mv[idx] * s[:, 0] / denom
        w2[idx, 1] = mv[idx] * s[:, 1] / denom
    return g_sel, w2.astype(np.float32)


def _pick_capacity(n: int):
    """Token capacity C and per-chunk widths. Chunks must be <=512 and
    multiples of 8 (16B SBUF lines); the first chunk is 16-aligned."""
    n = max(n, 64)
    nch = (n + 511) // 512
    c1 = -(-(-(-n // nch)) // 16) * 16
    cws = []
    rem = n
    for i in range(nch):
        w = min(c1, -(-rem // 8) * 8) if i == nch - 1 else c1
        cws.append(w)
        rem -= w
    cws = [w for w in cws if w > 0]
    return sum(cws), tuple(cws)


def _tile_gate_up(w):
    """[D, F] f32 -> dict of per-chunk [P, DO, w_i] bf16 in SBUF order."""
    t = w.reshape(DO, P, FFN_DIM).transpose(1, 0, 2)  # [P, DO, F]
    out = {}
    f0 = 0
    for i, wd in enumerate(FCHUNKS):
        out[i] = np.ascontiguousarray(t[:, :, f0 : f0 + wd]).astype(
            ml_dtypes.bfloat16
        )
        f0 += wd
    return out


def _tile_down(w):
    """[F, D] f32 -> [DJ, P, 2, FO, P] bf16 in SBUF order."""
    t = w.reshape(FO, P, DJ, 2, P).transpose(2, 1, 3, 0, 4)
    return np.ascontiguousarray(t).astype(ml_dtypes.bfloat16)


def _tile_x(xg, C):
    """[n, D] f32 -> [P, DO, C] bf16 (zero-padded to C tokens)."""
    out = np.zeros((P, DO, C), ml_dtypes.bfloat16)
    n = xg.shape[0]
    if n:
        # token c, feature d = do*P + p  ->  out[p, do, c]
        t = xg.T.reshape(DO, P, n).transpose(1, 0, 2)
        out[:, :, :n] = t.astype(ml_dtypes.bfloat16)
    return out


def kernel(x, macro_w, micro_w, gate_w, up_w, down_w):
    global LAST_RESULTS
    x = np.asarray(x)
    B, S, D = x.shape
    T = B * S
    xf = np.ascontiguousarray(x.reshape(T, D).astype(np.float32, copy=False))

    g_sel, w2 = _route(xf, np.asarray(macro_w), np.asarray(micro_w))
    idx_by_g = [np.nonzero(g_sel == g)[0] for g in range(NUM_GROUPS)]
    max_n = max(ix.size for ix in idx_by_g)

    n_rounds = max(1, math.ceil(max_n / C_CAP))
    if n_rounds > 1:
        C, cw = C_CAP, (512, 512)
    else:
        C, cw = _pick_capacity(max_n)
    nc = _get_program(C, cw)

    gate_w = np.asarray(gate_w, np.float32)
    up_w = np.asarray(up_w, np.float32)
    down_w = np.asarray(down_w, np.float32)
    gw_t = [_tile_gate_up(gate_w[e]) for e in range(NUM_EXPERTS)]
    uw_t = [_tile_gate_up(up_w[e]) for e in range(NUM_EXPERTS)]
    dw_t = [_tile_down(down_w[e]) for e in range(NUM_EXPERTS)]

    y = np.zeros((T, D), np.float32)
    for r in range(n_rounds):
        in_maps = []
        round_idx = []
        for c in range(N_CORES):
            g = c // 2
            j = c % 2  # local expert within group
            ix = idx_by_g[g][r * C_CAP : r * C_CAP + C]
            round_idx.append(ix)
            xg = xf[ix]
            wr = np.zeros((1, C), np.float32)
            wr[0, : ix.size] = w2[ix, j]
            im = {
                "xt": _tile_x(xg, C),
                "wrow": wr,
                "dw": dw_t[c],
            }
            for i in range(len(FCHUNKS)):
                im[f"gw{i}"] = gw_t[c][i]
                im[f"uw{i}"] = uw_t[c][i]
            in_maps.append(im)
        res = run_bass_kernel_spmd(nc, in_maps, core_ids=list(range(N_CORES)))
        LAST_RESULTS = res
        for g in range(NUM_GROUPS):
            ix = round_idx[2 * g]
            if ix.size:
                # yt: [DO, P, C] bf16 -> [D, C] f32
                y0 = res.results[2 * g]["yt"].astype(np.float32)
                y1 = res.results[2 * g + 1]["yt"].astype(np.float32)
                ysum = (y0 + y1).reshape(D, C)
                y[ix] = ysum[:, : ix.size].T
    return y.reshape(B, S, D)
